# revision 2
# baseline (speedup 1.0000x reference)
"""Trainium2 Bass kernel for Conv2d_XnorPP_SCA (binarized 3x3 conv).

Computes: out = conv2d(sign(x), round(tanh(w)), stride=1, pad=1) * alpha
  x: [64, 64, 112, 112] f32, w: [64, 64, 3, 3] f32, alpha: [64,1,1] f32

Strategy (per NeuronCore, data-parallel over batch, 8 images/core):
  - Zero-padded flat layout: each image is sign-binarized (bf16) into a
    [64, 114*114] SBUF region with zero borders; every 3x3 tap is then a
    constant column offset, so the conv is 9 PSUM-accumulated matmuls
    (K=Cin=64, M=Cout=64) per 4-row output group.
  - Two images are resident at once (partitions 0-63 / 64-127); both
    images of a group share ONE psum bank (each in its own partition
    half, own start=True region-clear), with the (row-group, col-group)
    quadrant assignment alternating by group parity so four matmul
    streams run concurrently on the four PE quadrants.
  - sign() runs on VectorE as a single bitwise op (int16 view of the
    f32 high halves: (x>>16 & 0x8000) | 0x3F80 == bf16 sign(x)),
    keeping ScalarE free for PSUM evacuation at full 128-partition
    width.
  - alpha is folded into the (ternary, exactly bf16-representable)
    weights.
"""

import numpy as np
import ml_dtypes

H = W = 112
WP = 114
P_COLS = WP * WP + 4  # 13000
CIN = COUT = 64
N_CORES = 8
NI = 8  # images per core
ROWS_PER_CHUNK = 56  # input load/sign granularity (2 chunks per pair)
GROUP_ROWS = 4  # output rows per psum bank
GROUPS_PER_SG = 14  # groups per output staging supergroup (56 rows)


def build_nc(ni=NI, reps=1):
    import concourse.bacc as bacc
    import concourse.mybir as mybir
    from concourse.tile import TileContext

    f32 = mybir.dt.float32
    bf16 = mybir.dt.bfloat16
    i16 = mybir.dt.int16

    nc = bacc.Bacc("TRN2", target_bir_lowering=False, debug=False)
    x_d = nc.dram_tensor("x", [ni, CIN, H, W], f32, kind="ExternalInput")
    w_d = nc.dram_tensor("w", [128, 9 * COUT], bf16, kind="ExternalInput")
    o_d = nc.dram_tensor("out", [ni, COUT, H, W], f32, kind="ExternalOutput")

    x_flat = x_d.ap().rearrange("n c h w -> (n c) (h w)")
    npairs = ni // 2
    n_chunks = H // ROWS_PER_CHUNK  # 2
    n_groups = H // GROUP_ROWS  # 28
    n_sg = n_groups // GROUPS_PER_SG  # 2

    with TileContext(nc) as tc:
        with (
            tc.tile_pool(name="wp", bufs=1) as wp,
            tc.tile_pool(name="inp", bufs=3) as inp,
            tc.tile_pool(name="pp", bufs=1) as pp,
            tc.tile_pool(name="op", bufs=2) as op,
            tc.tile_pool(name="psp", bufs=8, space="PSUM") as psp,
        ):
            w_sb = wp.tile([128, 9 * COUT], bf16, name="w_sb")
            nc.sync.dma_start(out=w_sb[:, :], in_=w_d.ap())

            p_tiles = []
            for i in range(2):
                pt = pp.tile([128, P_COLS], bf16, tag=f"p{i}", name=f"p{i}")
                nc.vector.memset(pt[:, :], 0.0)
                p_tiles.append(pt)

            loop = None
            if reps > 1:
                loop = tc.For_i(0, reps)
                loop.__enter__()

            for pair in range(npairs):
                p = p_tiles[pair % 2]
                # ---- load both images of the pair, binarize into p ----
                for ci in range(n_chunks):
                    y0 = ci * ROWS_PER_CHUNK
                    st = inp.tile([128, ROWS_PER_CHUNK * W], f32, tag="xin",
                                  name="xin")
                    nc.sync.dma_start(
                        out=st[:, :],
                        in_=x_flat[pair * 128:(pair + 1) * 128,
                                   y0 * W:(y0 + ROWS_PER_CHUNK) * W],
                    )
                    # sign(x) -> bf16 via bitwise trick on VectorE
                    src = st[:, :].bitcast(i16)
                    src = src.rearrange("q (n two) -> q n two", two=2)[:, :, 1]
                    src = src.rearrange("q (r w) -> q r w", w=W)
                    dst = p[:, 116 + y0 * WP: 116 + y0 * WP
                            + ROWS_PER_CHUNK * WP]
                    dst = dst.rearrange("q (r w) -> q r w", w=WP)[:, :, :W]
                    dst = dst.bitcast(i16)
                    nc.vector.tensor_scalar(
                        out=dst, in0=src,
                        scalar1=-32768.0, scalar2=float(0x3F80),
                        op0=mybir.AluOpType.bitwise_and,
                        op1=mybir.AluOpType.bitwise_or)

                # ---- conv: 28 groups of 4 output rows ----
                for sg in range(n_sg):
                    so = op.tile([128, GROUPS_PER_SG * GROUP_ROWS * W],
                                 f32, tag="so", name="so")
                    for g7 in range(GROUPS_PER_SG):
                        g = sg * GROUPS_PER_SG + g7
                        a = g % 2
                        bank = psp.tile([128, 456], f32, tag="ps", name="ps",
                                        padded_shape=[128, 512])
                        for t in range(9):
                            ky, kx = divmod(t, 3)
                            s = 116 + (4 * g + ky - 1) * WP + (kx - 1)
                            first, last = (t == 0), (t == 8)
                            for img in range(2):
                                cg = (img + a) % 2
                                nc.tensor.matmul(
                                    bank[64 * cg:64 * (cg + 1), 0:456],
                                    w_sb[64 * img:64 * (img + 1),
                                         t * 64:(t + 1) * 64],
                                    p[64 * img:64 * (img + 1), s:s + 456],
                                    start=first, stop=last)
                        # evacuate whole bank (both images) in one copy
                        src = bank[:, 0:456].rearrange(
                            "q (r w) -> q r w", w=WP)[:, :, :W]
                        dst = so[:, g7 * GROUP_ROWS * W:
                                 (g7 + 1) * GROUP_ROWS * W]
                        dst = dst.rearrange("q (r w) -> q r w", w=W)
                        nc.scalar.copy(out=dst, in_=src)
                    # ---- DMA out: 4 per (pair, sg) ----
                    # st partition half h in {0,1}; block g7: lower half
                    # holds img (g7%2), upper half img (1-g7%2); even g7
                    # blocks are rows 8*b2..+3 (rr=0), odd are rr=1.
                    y0 = sg * GROUPS_PER_SG * GROUP_ROWS
                    src4 = so[:, :].rearrange(
                        "q (b2 par w) -> q b2 par w", par=2,
                        w=GROUP_ROWS * W)
                    for h in range(2):
                        for parity in range(2):
                            img = (h + parity) % 2
                            n = pair * 2 + img
                            dsth = o_d.ap()[n, :, y0:y0 + 56, :].rearrange(
                                "c (b2 rr r) w -> c b2 rr (r w)",
                                rr=2, r=GROUP_ROWS)
                            nc.sync.dma_start(
                                out=dsth[:, :, parity],
                                in_=src4[64 * h:64 * (h + 1), :, parity])

            if loop is not None:
                loop.__exit__(None, None, None)
    nc.compile()
    return nc


def pack_weights(weight, alpha):
    """Ternarize (round(tanh(w))), fold alpha, pack as [128, 9*64] bf16 lhsT."""
    wt = _ternarize(np.asarray(weight, dtype=np.float32))
    wt = wt * np.asarray(alpha, dtype=np.float32).reshape(-1, 1, 1, 1)
    # lhsT[k=cin, t*64+cout]
    arr = wt.transpose(1, 2, 3, 0).reshape(CIN, 9 * COUT)
    pack = np.empty((128, 9 * COUT), dtype=ml_dtypes.bfloat16)
    pack[0:64] = arr.astype(ml_dtypes.bfloat16)
    pack[64:128] = pack[0:64]
    return pack


def _ternarize(w):
    try:
        import jax
        cpu = jax.devices("cpu")[0]
        with jax.default_device(cpu):
            import jax.numpy as jnp
            return np.asarray(jnp.round(jnp.tanh(jnp.asarray(w))))
    except Exception:
        return np.round(np.tanh(w.astype(np.float32))).astype(np.float32)


_NC_CACHE = {}


def _get_nc():
    if "nc" not in _NC_CACHE:
        _NC_CACHE["nc"] = build_nc(NI)
    return _NC_CACHE["nc"]


def _make_runner():
    """Build (once) a jitted shard_map callable running the NEFF on 8 cores."""
    if "runner" in _NC_CACHE:
        return _NC_CACHE["runner"]
    import jax
    import concourse.mybir as mybir
    from concourse import bass2jax
    from jax.sharding import Mesh, PartitionSpec
    from jax.experimental.shard_map import shard_map

    nc = _get_nc()
    bass2jax.install_neuronx_cc_hook()

    partition_name = (nc.partition_id_tensor.name
                      if nc.partition_id_tensor else None)
    in_names, out_names, out_avals, zero_shapes = [], [], [], []
    for alloc in nc.m.functions[0].allocations:
        if not isinstance(alloc, mybir.MemoryLocationSet):
            continue
        name = alloc.memorylocations[0].name
        if alloc.kind == "ExternalInput":
            if name != partition_name:
                in_names.append(name)
        elif alloc.kind == "ExternalOutput":
            out_names.append(name)
            shape = tuple(alloc.tensor_shape)
            dtype = mybir.dt.np(alloc.dtype)
            out_avals.append(jax.core.ShapedArray(shape, dtype))
            zero_shapes.append((shape, dtype))
    n_params = len(in_names)
    all_in_names = in_names + out_names
    if partition_name is not None:
        all_in_names = all_in_names + [partition_name]

    def _body(*args):
        operands = list(args)
        if partition_name is not None:
            operands.append(bass2jax.partition_id_tensor())
        outs = bass2jax._bass_exec_p.bind(
            *operands,
            out_avals=tuple(out_avals),
            in_names=tuple(all_in_names),
            out_names=tuple(out_names),
            lowering_input_output_aliases=(),
            sim_require_finite=True,
            sim_require_nnan=True,
            nc=nc,
        )
        return tuple(outs)

    devices = jax.devices()[:N_CORES]
    mesh = Mesh(np.asarray(devices), ("core",))
    n_outs = len(out_names)
    donate = tuple(range(n_params, n_params + n_outs))
    in_specs = (PartitionSpec("core"),) * (n_params + n_outs)
    out_specs = (PartitionSpec("core"),) * n_outs
    sharded = jax.jit(
        shard_map(_body, mesh=mesh, in_specs=in_specs, out_specs=out_specs,
                  check_rep=False),
        donate_argnums=donate, keep_unused=True)
    runner = {
        "fn": sharded, "mesh": mesh, "in_names": in_names,
        "out_names": out_names, "zero_shapes": zero_shapes,
        "n_params": n_params,
    }
    _NC_CACHE["runner"] = runner
    return runner


def make_concat_inputs(x, w_pack):
    """Per-core inputs concatenated on axis 0 (shard_map layout)."""
    xs = np.ascontiguousarray(x.reshape(N_CORES * NI, CIN, H, W))
    ws = np.concatenate([w_pack] * N_CORES, axis=0)
    return {"x": xs, "w": ws}


def make_zeros():
    r = _make_runner()
    return [np.zeros((N_CORES * s[0], *s[1:]), d) for s, d in r["zero_shapes"]]


def run_concat(concat_by_name, zeros=None):
    """Run on 8 cores. Inputs may be numpy or device-resident jax arrays."""
    r = _make_runner()
    if zeros is None:
        zeros = make_zeros()
    args = [concat_by_name[n] for n in r["in_names"]] + list(zeros)
    out_arrs = r["fn"](*args)
    return out_arrs


def kernel(x, weight, alpha):
    x = np.asarray(x, dtype=np.float32)
    w_pack = pack_weights(weight, alpha)
    concat = make_concat_inputs(x, w_pack)
    out_arrs = run_concat(concat)
    out = np.asarray(out_arrs[0]).reshape(64, COUT, H, W)
    return out.astype(np.float32, copy=False)


# revision 16
# speedup vs baseline: 10.0782x; 10.0782x over previous
"""Trainium2 Bass kernel for Conv2d_XnorPP_SCA (binarized 3x3 conv).

Computes: out = conv2d(sign(x), round(tanh(w)), stride=1, pad=1) * alpha
  x: [64, 64, 112, 112] f32, w: [64, 64, 3, 3] f32, alpha: [64,1,1] f32

Strategy (per NeuronCore, data-parallel over batch, 8 images/core):
  - Zero-padded flat layout: each image is sign-binarized (bf16) into a
    [64, 114*114] SBUF region with zero borders; every 3x3 tap is then a
    constant column offset, so the conv is 9 PSUM-accumulated matmuls
    (K=Cin=64, M=Cout=64) per 4-row output group.
  - Two images are resident at once (partitions 0-63 / 64-127); both
    images of a group share ONE psum bank (each in its own partition
    half, own start=True region-clear), with the (row-group, col-group)
    quadrant assignment alternating by group parity so four matmul
    streams run concurrently on the four PE quadrants.
  - sign() runs on VectorE as a single bitwise op (int16 view of the
    f32 high halves: (x>>16 & 0x8000) | 0x3F80 == bf16 sign(x)),
    keeping ScalarE free for PSUM evacuation at full 128-partition
    width.
  - alpha is folded into the (ternary, exactly bf16-representable)
    weights.
"""

import numpy as np
import ml_dtypes

H = W = 112
WP = 114
P_COLS = WP * WP + 4  # 13000
CIN = COUT = 64
N_CORES = 8
NI = 8  # images per core
ROWS_PER_CHUNK = 56  # input load/sign granularity (2 chunks per pair)
GROUP_ROWS = 4  # output rows per psum bank
GROUPS_PER_SG = 14  # groups per output staging supergroup (56 rows)


def build_nc(ni=NI, reps=1, ablate=()):
    import concourse.bacc as bacc
    import concourse.mybir as mybir
    from concourse.tile import TileContext

    f32 = mybir.dt.float32
    bf16 = mybir.dt.bfloat16
    i16 = mybir.dt.int16

    nc = bacc.Bacc("TRN2", target_bir_lowering=False, debug=False)
    x_d = nc.dram_tensor("x", [ni, CIN, H, W], f32, kind="ExternalInput")
    w_d = nc.dram_tensor("w", [128, 9 * COUT], bf16, kind="ExternalInput")
    o_d = nc.dram_tensor("out", [ni, COUT, H, W], f32, kind="ExternalOutput")

    x_flat = x_d.ap().rearrange("n c h w -> (n c) (h w)")
    npairs = ni // 2
    n_chunks = H // ROWS_PER_CHUNK  # 2
    n_groups = H // GROUP_ROWS  # 28
    n_sg = n_groups // GROUPS_PER_SG  # 2

    with TileContext(nc) as tc:
        with (
            tc.tile_pool(name="wp", bufs=1) as wp,
            tc.tile_pool(name="inp", bufs=3) as inp,
            tc.tile_pool(name="pp", bufs=1) as pp,
            tc.tile_pool(name="op", bufs=2) as op,
            tc.tile_pool(name="psp", bufs=4, space="PSUM") as psp,
        ):
            w_sb = wp.tile([128, 9 * COUT], bf16, name="w_sb")
            nc.sync.dma_start(out=w_sb[:, :], in_=w_d.ap())

            p_tiles = []
            for i in range(2):
                pt = pp.tile([128, P_COLS], bf16, tag=f"p{i}", name=f"p{i}")
                # zero only the padding borders (sign overwrites the rest):
                # head, per-row 2-col gaps, and tail below the last row.
                nc.vector.memset(pt[:, 0:116], 0.0)
                gaps = pt[:, 116:116 + H * WP].rearrange(
                    "q (r w) -> q r w", w=WP)[:, :, W:WP]
                nc.vector.memset(gaps, 0.0)
                nc.vector.memset(pt[:, 116 + H * WP:P_COLS], 0.0)
                p_tiles.append(pt)

            loop = None
            if reps > 1:
                loop = tc.For_i(0, reps)
                loop.__enter__()

            def load_chunk(pair, ci):
                """Issue input DMA + sign for one 56-row chunk of a pair."""
                p = p_tiles[pair % 2]
                y0 = ci * ROWS_PER_CHUNK
                st = inp.tile([128, ROWS_PER_CHUNK * W], f32, tag="xin",
                              name="xin")
                if "noin" not in ablate:
                    nc.sync.dma_start(
                        out=st[:, :],
                        in_=x_flat[pair * 128:(pair + 1) * 128,
                                   y0 * W:(y0 + ROWS_PER_CHUNK) * W],
                    )
                # sign(x) -> bf16 via bitwise trick on VectorE
                src = st[:, :].bitcast(i16)
                src = src.rearrange("q (n two) -> q n two", two=2)[:, :, 1]
                src = src.rearrange("q (r w) -> q r w", w=W)
                dst = p[:, 116 + y0 * WP: 116 + y0 * WP
                        + ROWS_PER_CHUNK * WP]
                dst = dst.rearrange("q (r w) -> q r w", w=WP)[:, :, :W]
                dst = dst.bitcast(i16)
                if "nosign" not in ablate:
                    nc.vector.tensor_scalar(
                        out=dst, in0=src,
                        scalar1=-32768.0, scalar2=float(0x3F80),
                        op0=mybir.AluOpType.bitwise_and,
                        op1=mybir.AluOpType.bitwise_or)

            for pair in range(npairs):
                p = p_tiles[pair % 2]
                if pair == 0:
                    for ci in range(n_chunks):
                        load_chunk(0, ci)

                # ---- conv: 28 groups of 4 output rows ----
                for sg in range(n_sg):
                    so = op.tile([128, GROUPS_PER_SG * GROUP_ROWS * W],
                                 f32, tag="so", name="so")
                    dbank = None
                    for g7 in range(GROUPS_PER_SG):
                        g = sg * GROUPS_PER_SG + g7
                        a = g % 2
                        b = g7 % 2
                        if b == 0:
                            dbank = psp.tile([128, 1024], f32, tag="ps",
                                             name="ps",
                                             padded_shape=[128, 1024])
                        for t in range(9):
                            ky, kx = divmod(t, 3)
                            s = 116 + (4 * g + ky - 1) * WP + (kx - 1)
                            first, last = (t == 0), (t == 8)
                            for img in range(2):
                                cg = (img + a) % 2
                                if "noconv" not in ablate:
                                    nc.tensor.matmul(
                                        dbank[64 * cg:64 * (cg + 1),
                                              512 * b:512 * b + 456],
                                        w_sb[64 * img:64 * (img + 1),
                                             t * 64:(t + 1) * 64],
                                        p[64 * img:64 * (img + 1),
                                          s:s + 456],
                                        start=first, stop=last)
                        if b == 1:
                            # evacuate both banks (both images) in one copy
                            gp = g7 // 2
                            src = dbank[:, 0:1024].rearrange(
                                "q (b k) -> q b k", b=2)[:, :, 0:456]
                            src = src.rearrange(
                                "q b (r w) -> q b r w", w=WP)[:, :, :, :W]
                            dst = so[:, gp * 2 * GROUP_ROWS * W:
                                     (gp + 1) * 2 * GROUP_ROWS * W]
                            dst = dst.rearrange("q (b r w) -> q b r w",
                                                b=2, w=W)
                            if "noevac" not in ablate:
                                if gp % 3 == 2:
                                    nc.vector.tensor_copy(out=dst, in_=src)
                                else:
                                    nc.scalar.copy(out=dst, in_=src)
                    # ---- DMA out: 4 per (pair, sg) ----
                    # st partition half h in {0,1}; block g7: lower half
                    # holds img (g7%2), upper half img (1-g7%2); even g7
                    # blocks are rows 8*b2..+3 (rr=0), odd are rr=1.
                    # prefetch next pair's chunk before issuing out-DMAs so
                    # the sync HWDGE ring never stalls input loads behind
                    # output DMAs waiting on evac.
                    if pair + 1 < npairs:
                        load_chunk(pair + 1, sg)
                    y0 = sg * GROUPS_PER_SG * GROUP_ROWS
                    src4 = so[:, :].rearrange(
                        "q (b2 par w) -> q b2 par w", par=2,
                        w=GROUP_ROWS * W)
                    for h in range(2):
                        for parity in range(2):
                            img = (h + parity) % 2
                            n = pair * 2 + img
                            dsth = o_d.ap()[n, :, y0:y0 + 56, :].rearrange(
                                "c (b2 rr r) w -> c b2 rr (r w)",
                                rr=2, r=GROUP_ROWS)
                            if "noout" not in ablate:
                                nc.sync.dma_start(
                                    out=dsth[:, :, parity],
                                    in_=src4[64 * h:64 * (h + 1), :, parity])

            if loop is not None:
                loop.__exit__(None, None, None)
    nc.compile()
    return nc


def pack_weights(weight, alpha):
    """Ternarize (round(tanh(w))), fold alpha, pack as [128, 9*64] bf16 lhsT."""
    wt = _ternarize(np.asarray(weight, dtype=np.float32))
    wt = wt * np.asarray(alpha, dtype=np.float32).reshape(-1, 1, 1, 1)
    # lhsT[k=cin, t*64+cout]
    arr = wt.transpose(1, 2, 3, 0).reshape(CIN, 9 * COUT)
    pack = np.empty((128, 9 * COUT), dtype=ml_dtypes.bfloat16)
    pack[0:64] = arr.astype(ml_dtypes.bfloat16)
    pack[64:128] = pack[0:64]
    return pack


def _ternarize(w):
    try:
        import jax
        cpu = jax.devices("cpu")[0]
        with jax.default_device(cpu):
            import jax.numpy as jnp
            return np.asarray(jnp.round(jnp.tanh(jnp.asarray(w))))
    except Exception:
        return np.round(np.tanh(w.astype(np.float32))).astype(np.float32)


_NC_CACHE = {}


def _get_nc():
    if "nc" not in _NC_CACHE:
        _NC_CACHE["nc"] = build_nc(NI)
    return _NC_CACHE["nc"]


def _make_runner():
    """Build (once) a jitted shard_map callable running the NEFF on 8 cores."""
    if "runner" in _NC_CACHE:
        return _NC_CACHE["runner"]
    import jax
    import concourse.mybir as mybir
    from concourse import bass2jax
    from jax.sharding import Mesh, PartitionSpec
    from jax.experimental.shard_map import shard_map

    nc = _get_nc()
    bass2jax.install_neuronx_cc_hook()

    partition_name = (nc.partition_id_tensor.name
                      if nc.partition_id_tensor else None)
    in_names, out_names, out_avals, zero_shapes = [], [], [], []
    for alloc in nc.m.functions[0].allocations:
        if not isinstance(alloc, mybir.MemoryLocationSet):
            continue
        name = alloc.memorylocations[0].name
        if alloc.kind == "ExternalInput":
            if name != partition_name:
                in_names.append(name)
        elif alloc.kind == "ExternalOutput":
            out_names.append(name)
            shape = tuple(alloc.tensor_shape)
            dtype = mybir.dt.np(alloc.dtype)
            out_avals.append(jax.core.ShapedArray(shape, dtype))
            zero_shapes.append((shape, dtype))
    n_params = len(in_names)
    all_in_names = in_names + out_names
    if partition_name is not None:
        all_in_names = all_in_names + [partition_name]

    def _body(*args):
        operands = list(args)
        if partition_name is not None:
            operands.append(bass2jax.partition_id_tensor())
        outs = bass2jax._bass_exec_p.bind(
            *operands,
            out_avals=tuple(out_avals),
            in_names=tuple(all_in_names),
            out_names=tuple(out_names),
            lowering_input_output_aliases=(),
            sim_require_finite=True,
            sim_require_nnan=True,
            nc=nc,
        )
        return tuple(outs)

    devices = jax.devices()[:N_CORES]
    mesh = Mesh(np.asarray(devices), ("core",))
    n_outs = len(out_names)
    donate = tuple(range(n_params, n_params + n_outs))
    in_specs = (PartitionSpec("core"),) * (n_params + n_outs)
    out_specs = (PartitionSpec("core"),) * n_outs
    sharded = jax.jit(
        shard_map(_body, mesh=mesh, in_specs=in_specs, out_specs=out_specs,
                  check_rep=False),
        donate_argnums=donate, keep_unused=True)
    runner = {
        "fn": sharded, "mesh": mesh, "in_names": in_names,
        "out_names": out_names, "zero_shapes": zero_shapes,
        "n_params": n_params,
    }
    _NC_CACHE["runner"] = runner
    return runner


def make_concat_inputs(x, w_pack):
    """Per-core inputs concatenated on axis 0 (shard_map layout)."""
    xs = np.ascontiguousarray(x.reshape(N_CORES * NI, CIN, H, W))
    ws = np.concatenate([w_pack] * N_CORES, axis=0)
    return {"x": xs, "w": ws}


def make_zeros():
    r = _make_runner()
    return [np.zeros((N_CORES * s[0], *s[1:]), d) for s, d in r["zero_shapes"]]


def _cached_zeros():
    if "zeros" not in _NC_CACHE:
        _NC_CACHE["zeros"] = make_zeros()
    return _NC_CACHE["zeros"]


def run_concat(concat_by_name, zeros=None):
    """Run on 8 cores. Inputs may be numpy or device-resident jax arrays."""
    r = _make_runner()
    if zeros is None:
        zeros = make_zeros()
    args = [concat_by_name[n] for n in r["in_names"]] + list(zeros)
    out_arrs = r["fn"](*args)
    return out_arrs


def kernel(x, weight, alpha):
    x = np.asarray(x, dtype=np.float32)
    w_pack = pack_weights(weight, alpha)
    concat = make_concat_inputs(x, w_pack)
    out_arrs = run_concat(concat, zeros=_cached_zeros())
    out = np.asarray(out_arrs[0]).reshape(64, COUT, H, W)
    return out.astype(np.float32, copy=False)


# revision 19
# speedup vs baseline: 10.3450x; 1.0265x over previous
"""Trainium2 Bass kernel for Conv2d_XnorPP_SCA (binarized 3x3 conv).

Computes: out = conv2d(sign(x), round(tanh(w)), stride=1, pad=1) * alpha
  x: [64, 64, 112, 112] f32, w: [64, 64, 3, 3] f32, alpha: [64,1,1] f32

Strategy (per NeuronCore, data-parallel over batch, 8 images/core):
  - Zero-padded flat layout: each image is sign-binarized (bf16) into a
    [64, 114*114] SBUF region with zero borders; every 3x3 tap is then a
    constant column offset, so the conv is 9 PSUM-accumulated matmuls
    (K=Cin=64, M=Cout=64) per 4-row output group.
  - Two images are resident at once (partitions 0-63 / 64-127); both
    images of a group share ONE psum bank (each in its own partition
    half, own start=True region-clear), with the (row-group, col-group)
    quadrant assignment alternating by group parity so four matmul
    streams run concurrently on the four PE quadrants.
  - sign() runs on VectorE as a single bitwise op (int16 view of the
    f32 high halves: (x>>16 & 0x8000) | 0x3F80 == bf16 sign(x)),
    keeping ScalarE free for PSUM evacuation at full 128-partition
    width.
  - alpha is folded into the (ternary, exactly bf16-representable)
    weights.
"""

import numpy as np
import ml_dtypes

H = W = 112
WP = 114
P_COLS = WP * WP + 4  # 13000
CIN = COUT = 64
N_CORES = 8
NI = 8  # images per core
ROWS_PER_CHUNK = 56  # input load/sign granularity (2 chunks per pair)
GROUP_ROWS = 4  # output rows per psum bank
GROUPS_PER_SG = 14  # groups per output staging supergroup (56 rows)


def build_nc(ni=NI, reps=1, ablate=()):
    import concourse.bacc as bacc
    import concourse.mybir as mybir
    from concourse.tile import TileContext

    f32 = mybir.dt.float32
    bf16 = mybir.dt.bfloat16
    i16 = mybir.dt.int16

    nc = bacc.Bacc("TRN2", target_bir_lowering=False, debug=False)
    x_d = nc.dram_tensor("x", [ni, CIN, H, W], f32, kind="ExternalInput")
    w_d = nc.dram_tensor("w", [128, 9 * COUT], bf16, kind="ExternalInput")
    o_d = nc.dram_tensor("out", [ni, COUT, H, W], f32, kind="ExternalOutput")

    x_flat = x_d.ap().rearrange("n c h w -> (n c) (h w)")
    npairs = ni // 2
    n_chunks = H // ROWS_PER_CHUNK  # 2
    n_groups = H // GROUP_ROWS  # 28
    n_sg = n_groups // GROUPS_PER_SG  # 2

    with TileContext(nc) as tc:
        with (
            tc.tile_pool(name="wp", bufs=1) as wp,
            tc.tile_pool(name="inp", bufs=3) as inp,
            tc.tile_pool(name="pp", bufs=1) as pp,
            tc.tile_pool(name="op", bufs=3) as op,
            tc.tile_pool(name="psp", bufs=4, space="PSUM") as psp,
        ):
            w_sb = wp.tile([128, 9 * COUT], bf16, name="w_sb")
            nc.sync.dma_start(out=w_sb[:, :], in_=w_d.ap())

            p_tiles = []
            for i in range(2):
                pt = pp.tile([128, P_COLS], bf16, tag=f"p{i}", name=f"p{i}")
                # zero only the padding borders (sign overwrites the rest):
                # head, per-row 2-col gaps, and tail below the last row.
                nc.vector.memset(pt[:, 0:116], 0.0)
                gaps = pt[:, 116:116 + H * WP].rearrange(
                    "q (r w) -> q r w", w=WP)[:, :, W:WP]
                nc.vector.memset(gaps, 0.0)
                nc.vector.memset(pt[:, 116 + H * WP:P_COLS], 0.0)
                p_tiles.append(pt)

            loop = None
            if reps > 1:
                loop = tc.For_i(0, reps)
                loop.__enter__()

            def load_chunk(pair, ci):
                """Issue input DMA + sign for one 56-row chunk of a pair."""
                p = p_tiles[pair % 2]
                y0 = ci * ROWS_PER_CHUNK
                st = inp.tile([128, ROWS_PER_CHUNK * W], f32, tag="xin",
                              name="xin")
                if "noin" not in ablate:
                    nc.sync.dma_start(
                        out=st[:, :],
                        in_=x_flat[pair * 128:(pair + 1) * 128,
                                   y0 * W:(y0 + ROWS_PER_CHUNK) * W],
                    )
                # sign(x) -> bf16 via bitwise trick on VectorE
                src = st[:, :].bitcast(i16)
                src = src.rearrange("q (n two) -> q n two", two=2)[:, :, 1]
                src = src.rearrange("q (r w) -> q r w", w=W)
                dst = p[:, 116 + y0 * WP: 116 + y0 * WP
                        + ROWS_PER_CHUNK * WP]
                dst = dst.rearrange("q (r w) -> q r w", w=WP)[:, :, :W]
                dst = dst.bitcast(i16)
                if "nosign" not in ablate:
                    nc.vector.tensor_scalar(
                        out=dst, in0=src,
                        scalar1=-32768.0, scalar2=float(0x3F80),
                        op0=mybir.AluOpType.bitwise_and,
                        op1=mybir.AluOpType.bitwise_or)

            for pair in range(npairs):
                p = p_tiles[pair % 2]
                if pair == 0:
                    for ci in range(n_chunks):
                        load_chunk(0, ci)

                # ---- conv: 28 groups of 4 output rows ----
                for sg in range(n_sg):
                    so = op.tile([128, GROUPS_PER_SG * GROUP_ROWS * W],
                                 f32, tag="so", name="so")
                    dbank = None
                    for g7 in range(GROUPS_PER_SG):
                        g = sg * GROUPS_PER_SG + g7
                        a = g % 2
                        b = g7 % 2
                        if b == 0:
                            dbank = psp.tile([128, 1024], f32, tag="ps",
                                             name="ps",
                                             padded_shape=[128, 1024])
                        for t in range(9):
                            ky, kx = divmod(t, 3)
                            s = 116 + (4 * g + ky - 1) * WP + (kx - 1)
                            first, last = (t == 0), (t == 8)
                            for img in range(2):
                                cg = (img + a) % 2
                                if "noconv" not in ablate:
                                    nc.tensor.matmul(
                                        dbank[64 * cg:64 * (cg + 1),
                                              512 * b:512 * b + 456],
                                        w_sb[64 * img:64 * (img + 1),
                                             t * 64:(t + 1) * 64],
                                        p[64 * img:64 * (img + 1),
                                          s:s + 456],
                                        start=first, stop=last)
                        if b == 1:
                            # evacuate both banks (both images) in one copy
                            gp = g7 // 2
                            src = dbank[:, 0:1024].rearrange(
                                "q (b k) -> q b k", b=2)[:, :, 0:456]
                            src = src.rearrange(
                                "q b (r w) -> q b r w", w=WP)[:, :, :, :W]
                            dst = so[:, gp * 2 * GROUP_ROWS * W:
                                     (gp + 1) * 2 * GROUP_ROWS * W]
                            dst = dst.rearrange("q (b r w) -> q b r w",
                                                b=2, w=W)
                            if "noevac" not in ablate:
                                if gp % 3 == 2:
                                    nc.vector.tensor_copy(out=dst, in_=src)
                                else:
                                    nc.scalar.copy(out=dst, in_=src)
                    # ---- DMA out: 4 per (pair, sg) ----
                    # st partition half h in {0,1}; block g7: lower half
                    # holds img (g7%2), upper half img (1-g7%2); even g7
                    # blocks are rows 8*b2..+3 (rr=0), odd are rr=1.
                    # prefetch next pair's chunk before issuing out-DMAs so
                    # the sync HWDGE ring never stalls input loads behind
                    # output DMAs waiting on evac.
                    if pair + 1 < npairs:
                        load_chunk(pair + 1, sg)
                    y0 = sg * GROUPS_PER_SG * GROUP_ROWS
                    src4 = so[:, :].rearrange(
                        "q (b2 par w) -> q b2 par w", par=2,
                        w=GROUP_ROWS * W)
                    for parity in range(2):
                        for h in range(2):
                            img = (h + parity) % 2
                            n = pair * 2 + img
                            dsth = o_d.ap()[n, :, y0:y0 + 56, :].rearrange(
                                "c (b2 rr r) w -> c b2 rr (r w)",
                                rr=2, r=GROUP_ROWS)
                            if "noout" not in ablate:
                                nc.sync.dma_start(
                                    out=dsth[:, :, parity],
                                    in_=src4[64 * h:64 * (h + 1), :, parity])

            if loop is not None:
                loop.__exit__(None, None, None)
    nc.compile()
    return nc


def pack_weights(weight, alpha):
    """Ternarize (round(tanh(w))), fold alpha, pack as [128, 9*64] bf16 lhsT."""
    wt = _ternarize(np.asarray(weight, dtype=np.float32))
    wt = wt * np.asarray(alpha, dtype=np.float32).reshape(-1, 1, 1, 1)
    # lhsT[k=cin, t*64+cout]
    arr = wt.transpose(1, 2, 3, 0).reshape(CIN, 9 * COUT)
    pack = np.empty((128, 9 * COUT), dtype=ml_dtypes.bfloat16)
    pack[0:64] = arr.astype(ml_dtypes.bfloat16)
    pack[64:128] = pack[0:64]
    return pack


def _ternarize(w):
    try:
        import jax
        cpu = jax.devices("cpu")[0]
        with jax.default_device(cpu):
            import jax.numpy as jnp
            return np.asarray(jnp.round(jnp.tanh(jnp.asarray(w))))
    except Exception:
        return np.round(np.tanh(w.astype(np.float32))).astype(np.float32)


_NC_CACHE = {}


def _get_nc():
    if "nc" not in _NC_CACHE:
        _NC_CACHE["nc"] = build_nc(NI)
    return _NC_CACHE["nc"]


def _make_runner():
    """Build (once) a jitted shard_map callable running the NEFF on 8 cores."""
    if "runner" in _NC_CACHE:
        return _NC_CACHE["runner"]
    import jax
    import concourse.mybir as mybir
    from concourse import bass2jax
    from jax.sharding import Mesh, PartitionSpec
    from jax.experimental.shard_map import shard_map

    nc = _get_nc()
    bass2jax.install_neuronx_cc_hook()

    partition_name = (nc.partition_id_tensor.name
                      if nc.partition_id_tensor else None)
    in_names, out_names, out_avals, zero_shapes = [], [], [], []
    for alloc in nc.m.functions[0].allocations:
        if not isinstance(alloc, mybir.MemoryLocationSet):
            continue
        name = alloc.memorylocations[0].name
        if alloc.kind == "ExternalInput":
            if name != partition_name:
                in_names.append(name)
        elif alloc.kind == "ExternalOutput":
            out_names.append(name)
            shape = tuple(alloc.tensor_shape)
            dtype = mybir.dt.np(alloc.dtype)
            out_avals.append(jax.core.ShapedArray(shape, dtype))
            zero_shapes.append((shape, dtype))
    n_params = len(in_names)
    all_in_names = in_names + out_names
    if partition_name is not None:
        all_in_names = all_in_names + [partition_name]

    def _body(*args):
        operands = list(args)
        if partition_name is not None:
            operands.append(bass2jax.partition_id_tensor())
        outs = bass2jax._bass_exec_p.bind(
            *operands,
            out_avals=tuple(out_avals),
            in_names=tuple(all_in_names),
            out_names=tuple(out_names),
            lowering_input_output_aliases=(),
            sim_require_finite=True,
            sim_require_nnan=True,
            nc=nc,
        )
        return tuple(outs)

    devices = jax.devices()[:N_CORES]
    mesh = Mesh(np.asarray(devices), ("core",))
    n_outs = len(out_names)
    donate = tuple(range(n_params, n_params + n_outs))
    in_specs = (PartitionSpec("core"),) * (n_params + n_outs)
    out_specs = (PartitionSpec("core"),) * n_outs
    sharded = jax.jit(
        shard_map(_body, mesh=mesh, in_specs=in_specs, out_specs=out_specs,
                  check_rep=False),
        donate_argnums=donate, keep_unused=True)
    runner = {
        "fn": sharded, "mesh": mesh, "in_names": in_names,
        "out_names": out_names, "zero_shapes": zero_shapes,
        "n_params": n_params,
    }
    _NC_CACHE["runner"] = runner
    return runner


def make_concat_inputs(x, w_pack):
    """Per-core inputs concatenated on axis 0 (shard_map layout)."""
    xs = np.ascontiguousarray(x.reshape(N_CORES * NI, CIN, H, W))
    ws = np.concatenate([w_pack] * N_CORES, axis=0)
    return {"x": xs, "w": ws}


def make_zeros():
    r = _make_runner()
    return [np.zeros((N_CORES * s[0], *s[1:]), d) for s, d in r["zero_shapes"]]


def _cached_zeros():
    if "zeros" not in _NC_CACHE:
        _NC_CACHE["zeros"] = make_zeros()
    return _NC_CACHE["zeros"]


def run_concat(concat_by_name, zeros=None):
    """Run on 8 cores. Inputs may be numpy or device-resident jax arrays."""
    r = _make_runner()
    if zeros is None:
        zeros = make_zeros()
    args = [concat_by_name[n] for n in r["in_names"]] + list(zeros)
    out_arrs = r["fn"](*args)
    return out_arrs


def kernel(x, weight, alpha):
    x = np.asarray(x, dtype=np.float32)
    w_pack = pack_weights(weight, alpha)
    concat = make_concat_inputs(x, w_pack)
    out_arrs = run_concat(concat, zeros=_cached_zeros())
    out = np.asarray(out_arrs[0]).reshape(64, COUT, H, W)
    return out.astype(np.float32, copy=False)


# revision 20
# speedup vs baseline: 10.8030x; 1.0443x over previous
"""Trainium2 Bass kernel for Conv2d_XnorPP_SCA (binarized 3x3 conv).

Computes: out = conv2d(sign(x), round(tanh(w)), stride=1, pad=1) * alpha
  x: [64, 64, 112, 112] f32, w: [64, 64, 3, 3] f32, alpha: [64,1,1] f32

Strategy (per NeuronCore, data-parallel over batch, 8 images/core):
  - Zero-padded flat layout: each image is sign-binarized (bf16) into a
    [64, 114*114] SBUF region with zero borders; every 3x3 tap is then a
    constant column offset, so the conv is 9 PSUM-accumulated matmuls
    (K=Cin=64, M=Cout=64) per 4-row output group.
  - Two images are resident at once (partitions 0-63 / 64-127); both
    images of a group share ONE psum bank (each in its own partition
    half, own start=True region-clear), with the (row-group, col-group)
    quadrant assignment alternating by group parity so four matmul
    streams run concurrently on the four PE quadrants.
  - sign() runs on VectorE as a single bitwise op (int16 view of the
    f32 high halves: (x>>16 & 0x8000) | 0x3F80 == bf16 sign(x)),
    keeping ScalarE free for PSUM evacuation at full 128-partition
    width.
  - alpha is folded into the (ternary, exactly bf16-representable)
    weights.
"""

import numpy as np
import ml_dtypes

H = W = 112
WP = 114
P_COLS = WP * WP + 4  # 13000
CIN = COUT = 64
N_CORES = 8
NI = 8  # images per core
ROWS_PER_CHUNK = 28  # input load/sign granularity (4 chunks per pair)
GROUP_ROWS = 4  # output rows per psum bank
GROUPS_PER_SG = 14  # groups per output staging supergroup (56 rows)


def build_nc(ni=NI, reps=1, ablate=()):
    import concourse.bacc as bacc
    import concourse.mybir as mybir
    from concourse.tile import TileContext

    f32 = mybir.dt.float32
    bf16 = mybir.dt.bfloat16
    i16 = mybir.dt.int16

    nc = bacc.Bacc("TRN2", target_bir_lowering=False, debug=False)
    x_d = nc.dram_tensor("x", [ni, CIN, H, W], f32, kind="ExternalInput")
    w_d = nc.dram_tensor("w", [128, 9 * COUT], bf16, kind="ExternalInput")
    o_d = nc.dram_tensor("out", [ni, COUT, H, W], f32, kind="ExternalOutput")

    x_flat = x_d.ap().rearrange("n c h w -> (n c) (h w)")
    npairs = ni // 2
    n_chunks = H // ROWS_PER_CHUNK  # 2
    n_groups = H // GROUP_ROWS  # 28
    n_sg = n_groups // GROUPS_PER_SG  # 2

    with TileContext(nc) as tc:
        with (
            tc.tile_pool(name="wp", bufs=1) as wp,
            tc.tile_pool(name="inp", bufs=6) as inp,
            tc.tile_pool(name="pp", bufs=1) as pp,
            tc.tile_pool(name="op", bufs=3) as op,
            tc.tile_pool(name="psp", bufs=4, space="PSUM") as psp,
        ):
            w_sb = wp.tile([128, 9 * COUT], bf16, name="w_sb")
            nc.sync.dma_start(out=w_sb[:, :], in_=w_d.ap())

            p_tiles = []
            for i in range(2):
                pt = pp.tile([128, P_COLS], bf16, tag=f"p{i}", name=f"p{i}")
                # zero only the padding borders (sign overwrites the rest):
                # head, per-row 2-col gaps, and tail below the last row.
                nc.vector.memset(pt[:, 0:116], 0.0)
                gaps = pt[:, 116:116 + H * WP].rearrange(
                    "q (r w) -> q r w", w=WP)[:, :, W:WP]
                nc.vector.memset(gaps, 0.0)
                nc.vector.memset(pt[:, 116 + H * WP:P_COLS], 0.0)
                p_tiles.append(pt)

            loop = None
            if reps > 1:
                loop = tc.For_i(0, reps)
                loop.__enter__()

            def load_chunk(pair, ci):
                """Issue input DMA + sign for one 56-row chunk of a pair."""
                p = p_tiles[pair % 2]
                y0 = ci * ROWS_PER_CHUNK
                st = inp.tile([128, ROWS_PER_CHUNK * W], f32, tag="xin",
                              name="xin")
                if "noin" not in ablate:
                    nc.sync.dma_start(
                        out=st[:, :],
                        in_=x_flat[pair * 128:(pair + 1) * 128,
                                   y0 * W:(y0 + ROWS_PER_CHUNK) * W],
                    )
                # sign(x) -> bf16 via bitwise trick on VectorE
                src = st[:, :].bitcast(i16)
                src = src.rearrange("q (n two) -> q n two", two=2)[:, :, 1]
                src = src.rearrange("q (r w) -> q r w", w=W)
                dst = p[:, 116 + y0 * WP: 116 + y0 * WP
                        + ROWS_PER_CHUNK * WP]
                dst = dst.rearrange("q (r w) -> q r w", w=WP)[:, :, :W]
                dst = dst.bitcast(i16)
                if "nosign" not in ablate:
                    nc.vector.tensor_scalar(
                        out=dst, in0=src,
                        scalar1=-32768.0, scalar2=float(0x3F80),
                        op0=mybir.AluOpType.bitwise_and,
                        op1=mybir.AluOpType.bitwise_or)

            for pair in range(npairs):
                p = p_tiles[pair % 2]
                if pair == 0:
                    for ci in range(n_chunks):
                        load_chunk(0, ci)

                # ---- conv: 28 groups of 4 output rows ----
                for sg in range(n_sg):
                    so = op.tile([128, GROUPS_PER_SG * GROUP_ROWS * W],
                                 f32, tag="so", name="so")
                    dbank = None
                    for g7 in range(GROUPS_PER_SG):
                        g = sg * GROUPS_PER_SG + g7
                        a = g % 2
                        b = g7 % 2
                        if b == 0:
                            dbank = psp.tile([128, 1024], f32, tag="ps",
                                             name="ps",
                                             padded_shape=[128, 1024])
                        for t in range(9):
                            ky, kx = divmod(t, 3)
                            s = 116 + (4 * g + ky - 1) * WP + (kx - 1)
                            first, last = (t == 0), (t == 8)
                            for img in range(2):
                                cg = (img + a) % 2
                                if "noconv" not in ablate:
                                    nc.tensor.matmul(
                                        dbank[64 * cg:64 * (cg + 1),
                                              512 * b:512 * b + 456],
                                        w_sb[64 * img:64 * (img + 1),
                                             t * 64:(t + 1) * 64],
                                        p[64 * img:64 * (img + 1),
                                          s:s + 456],
                                        start=first, stop=last)
                        if b == 1:
                            # evacuate both banks (both images) in one copy
                            gp = g7 // 2
                            src = dbank[:, 0:1024].rearrange(
                                "q (b k) -> q b k", b=2)[:, :, 0:456]
                            src = src.rearrange(
                                "q b (r w) -> q b r w", w=WP)[:, :, :, :W]
                            dst = so[:, gp * 2 * GROUP_ROWS * W:
                                     (gp + 1) * 2 * GROUP_ROWS * W]
                            dst = dst.rearrange("q (b r w) -> q b r w",
                                                b=2, w=W)
                            if "noevac" not in ablate:
                                if gp % 3 == 2:
                                    nc.vector.tensor_copy(out=dst, in_=src)
                                else:
                                    nc.scalar.copy(out=dst, in_=src)
                    # ---- DMA out: 4 per (pair, sg) ----
                    # st partition half h in {0,1}; block g7: lower half
                    # holds img (g7%2), upper half img (1-g7%2); even g7
                    # blocks are rows 8*b2..+3 (rr=0), odd are rr=1.
                    # prefetch next pair's chunks before issuing out-DMAs so
                    # the sync HWDGE ring never stalls input loads behind
                    # output DMAs waiting on evac.
                    if pair + 1 < npairs:
                        load_chunk(pair + 1, sg * 2)
                        load_chunk(pair + 1, sg * 2 + 1)
                    y0 = sg * GROUPS_PER_SG * GROUP_ROWS
                    src4 = so[:, :].rearrange(
                        "q (b2 par w) -> q b2 par w", par=2,
                        w=GROUP_ROWS * W)
                    for parity in range(2):
                        for h in range(2):
                            img = (h + parity) % 2
                            n = pair * 2 + img
                            dsth = o_d.ap()[n, :, y0:y0 + 56, :].rearrange(
                                "c (b2 rr r) w -> c b2 rr (r w)",
                                rr=2, r=GROUP_ROWS)
                            if "noout" not in ablate:
                                nc.sync.dma_start(
                                    out=dsth[:, :, parity],
                                    in_=src4[64 * h:64 * (h + 1), :, parity])

            if loop is not None:
                loop.__exit__(None, None, None)
    nc.compile()
    return nc


def pack_weights(weight, alpha):
    """Ternarize (round(tanh(w))), fold alpha, pack as [128, 9*64] bf16 lhsT."""
    wt = _ternarize(np.asarray(weight, dtype=np.float32))
    wt = wt * np.asarray(alpha, dtype=np.float32).reshape(-1, 1, 1, 1)
    # lhsT[k=cin, t*64+cout]
    arr = wt.transpose(1, 2, 3, 0).reshape(CIN, 9 * COUT)
    pack = np.empty((128, 9 * COUT), dtype=ml_dtypes.bfloat16)
    pack[0:64] = arr.astype(ml_dtypes.bfloat16)
    pack[64:128] = pack[0:64]
    return pack


def _ternarize(w):
    try:
        import jax
        cpu = jax.devices("cpu")[0]
        with jax.default_device(cpu):
            import jax.numpy as jnp
            return np.asarray(jnp.round(jnp.tanh(jnp.asarray(w))))
    except Exception:
        return np.round(np.tanh(w.astype(np.float32))).astype(np.float32)


_NC_CACHE = {}


def _get_nc():
    if "nc" not in _NC_CACHE:
        _NC_CACHE["nc"] = build_nc(NI)
    return _NC_CACHE["nc"]


def _make_runner():
    """Build (once) a jitted shard_map callable running the NEFF on 8 cores."""
    if "runner" in _NC_CACHE:
        return _NC_CACHE["runner"]
    import jax
    import concourse.mybir as mybir
    from concourse import bass2jax
    from jax.sharding import Mesh, PartitionSpec
    from jax.experimental.shard_map import shard_map

    nc = _get_nc()
    bass2jax.install_neuronx_cc_hook()

    partition_name = (nc.partition_id_tensor.name
                      if nc.partition_id_tensor else None)
    in_names, out_names, out_avals, zero_shapes = [], [], [], []
    for alloc in nc.m.functions[0].allocations:
        if not isinstance(alloc, mybir.MemoryLocationSet):
            continue
        name = alloc.memorylocations[0].name
        if alloc.kind == "ExternalInput":
            if name != partition_name:
                in_names.append(name)
        elif alloc.kind == "ExternalOutput":
            out_names.append(name)
            shape = tuple(alloc.tensor_shape)
            dtype = mybir.dt.np(alloc.dtype)
            out_avals.append(jax.core.ShapedArray(shape, dtype))
            zero_shapes.append((shape, dtype))
    n_params = len(in_names)
    all_in_names = in_names + out_names
    if partition_name is not None:
        all_in_names = all_in_names + [partition_name]

    def _body(*args):
        operands = list(args)
        if partition_name is not None:
            operands.append(bass2jax.partition_id_tensor())
        outs = bass2jax._bass_exec_p.bind(
            *operands,
            out_avals=tuple(out_avals),
            in_names=tuple(all_in_names),
            out_names=tuple(out_names),
            lowering_input_output_aliases=(),
            sim_require_finite=True,
            sim_require_nnan=True,
            nc=nc,
        )
        return tuple(outs)

    devices = jax.devices()[:N_CORES]
    mesh = Mesh(np.asarray(devices), ("core",))
    n_outs = len(out_names)
    donate = tuple(range(n_params, n_params + n_outs))
    in_specs = (PartitionSpec("core"),) * (n_params + n_outs)
    out_specs = (PartitionSpec("core"),) * n_outs
    sharded = jax.jit(
        shard_map(_body, mesh=mesh, in_specs=in_specs, out_specs=out_specs,
                  check_rep=False),
        donate_argnums=donate, keep_unused=True)
    runner = {
        "fn": sharded, "mesh": mesh, "in_names": in_names,
        "out_names": out_names, "zero_shapes": zero_shapes,
        "n_params": n_params,
    }
    _NC_CACHE["runner"] = runner
    return runner


def make_concat_inputs(x, w_pack):
    """Per-core inputs concatenated on axis 0 (shard_map layout)."""
    xs = np.ascontiguousarray(x.reshape(N_CORES * NI, CIN, H, W))
    ws = np.concatenate([w_pack] * N_CORES, axis=0)
    return {"x": xs, "w": ws}


def make_zeros():
    r = _make_runner()
    return [np.zeros((N_CORES * s[0], *s[1:]), d) for s, d in r["zero_shapes"]]


def _cached_zeros():
    if "zeros" not in _NC_CACHE:
        _NC_CACHE["zeros"] = make_zeros()
    return _NC_CACHE["zeros"]


def run_concat(concat_by_name, zeros=None):
    """Run on 8 cores. Inputs may be numpy or device-resident jax arrays."""
    r = _make_runner()
    if zeros is None:
        zeros = make_zeros()
    args = [concat_by_name[n] for n in r["in_names"]] + list(zeros)
    out_arrs = r["fn"](*args)
    return out_arrs


def kernel(x, weight, alpha):
    x = np.asarray(x, dtype=np.float32)
    w_pack = pack_weights(weight, alpha)
    concat = make_concat_inputs(x, w_pack)
    out_arrs = run_concat(concat, zeros=_cached_zeros())
    out = np.asarray(out_arrs[0]).reshape(64, COUT, H, W)
    return out.astype(np.float32, copy=False)
